# revision 2
# baseline (speedup 1.0000x reference)
"""Trainium2 Bass kernel for nn_CNNStateEncoder (dense_cnn).

Network per row (B*S rows, 8 features each):
  conv1 2x2 on [1,2,4] -> 32ch x [1,3]   == h1[96]  = A1[96,8]  @ x[8],  relu(+b1)
  conv2 1x2 on [32,1,3] -> 32ch x [1,2]  == h2[64]  = A2[64,96] @ h1,    relu(+b2)
  linear 64->64                          == out[64] = Wp[64,64] @ h2 + bp

Mapping on each NeuronCore (data parallel over 8 cores, 65536 rows/core,
2048-row tiles; PE HAM stays at 1.2GHz on this part, so minimize matmul
count and maximize row/col-group concurrency):
  - rows live in the matmul free dim (feature-major chain)
  - input: DVE cast f32->bf16, gpsimd x4-replicate into 32-blocks, DVE
    StreamTranspose; row-chunk q's 8 features land at partitions 32q..32q+8
  - conv1: 4 concurrently-packed K=8 matmuls (row groups), one psum bank
    each (concurrent drains must hit distinct banks)
  - relu1: ONE contiguous ACT op over the 4 banks
  - conv2: K=96, N=512 matmuls; the tile's two 1024-row halves go to output
    col groups 0/64 and run concurrently (packed by partition halves)
  - linear: lhsT = activations (M=rows) -> row-major PSUM; 16 chunks issued
    as concurrent (row-group 0-1 x bank0, row-group 2-3 x bank1) pairs
  - out: single DVE bias-add+copy, single 512KB store
"""

import numpy as np
import ml_dtypes

B, S, FEAT, OUT = 64, 8192, 8, 64
NCORES = 8
ROWS_TOTAL = B * S
ROWS_CORE = ROWS_TOTAL // NCORES  # 65536
TILE_ROWS = 2048

BF16 = ml_dtypes.bfloat16

# ---------------------------------------------------------------------------
# numpy-side weight packing
# ---------------------------------------------------------------------------

def pack_weights(W1, b1, W2, b2, Wp, bp):
    W1 = np.asarray(W1, np.float32)
    W2 = np.asarray(W2, np.float32)
    Wp = np.asarray(Wp, np.float32)
    b1 = np.asarray(b1, np.float32)
    b2 = np.asarray(b2, np.float32)
    bp = np.asarray(bp, np.float32)

    # A1 [96, 8]: h1[o*3+j] = sum_{kh,kw} x[kh*4 + j + kw] * W1[o,0,kh,kw]
    A1 = np.zeros((96, 8), np.float32)
    for o in range(32):
        for j in range(3):
            for kh in range(2):
                for kw in range(2):
                    A1[o * 3 + j, kh * 4 + j + kw] += W1[o, 0, kh, kw]
    b1_96 = np.repeat(b1, 3).astype(np.float32)

    # A2 [64, 96]: h2[c*2+w] = sum_{i,kw} h1[i*3 + w + kw] * W2[c,i,0,kw]
    A2 = np.zeros((64, 96), np.float32)
    for c in range(32):
        for w in range(2):
            for i in range(32):
                for kw in range(2):
                    A2[c * 2 + w, i * 3 + w + kw] += W2[c, i, 0, kw]
    b2_64 = np.repeat(b2, 2).astype(np.float32)

    a1t = np.zeros((128, 96), np.float32)
    for q in range(4):
        a1t[32 * q:32 * q + 8, :] = A1.T
    a2t = np.zeros((96, 128), np.float32)
    a2t[:, 0:64] = A2.T
    a2t[:, 64:128] = A2.T
    wpt = np.zeros((128, 64), np.float32)
    wpt[0:64, :] = Wp.T
    wpt[64:128, :] = Wp.T
    b1c = b1_96.reshape(96, 1)
    b2c = np.concatenate([b2_64, b2_64]).reshape(128, 1)
    bpb = np.tile(bp, (128, TILE_ROWS // 128))  # [128, 1024]

    return {
        "a1t": a1t.astype(BF16),
        "a2t": a2t.astype(BF16),
        "wpt": wpt.astype(BF16),
        "b1c": b1c,
        "b2c": b2c,
        "bpb": bpb.astype(np.float32),
    }


# ---------------------------------------------------------------------------
# bass module
# ---------------------------------------------------------------------------

def build_nc(rows=ROWS_CORE):
    import concourse.bass as bass
    import concourse.bacc as bacc
    import concourse.mybir as mybir
    import concourse.tile as tile

    f32 = mybir.dt.float32
    bf16 = mybir.dt.bfloat16
    Relu = mybir.ActivationFunctionType.Relu
    Alu = mybir.AluOpType

    assert rows % TILE_ROWS == 0
    ntiles = rows // TILE_ROWS

    nc = bacc.Bacc(None, target_bir_lowering=False)

    x_d = nc.dram_tensor("x", [rows, FEAT], f32, kind="ExternalInput")
    a1t_d = nc.dram_tensor("a1t", [128, 96], bf16, kind="ExternalInput")
    a2t_d = nc.dram_tensor("a2t", [96, 128], bf16, kind="ExternalInput")
    wpt_d = nc.dram_tensor("wpt", [128, 64], bf16, kind="ExternalInput")
    b1c_d = nc.dram_tensor("b1c", [96, 1], f32, kind="ExternalInput")
    b2c_d = nc.dram_tensor("b2c", [128, 1], f32, kind="ExternalInput")
    bpb_d = nc.dram_tensor("bpb", [128, 1024], f32, kind="ExternalInput")
    out_d = nc.dram_tensor("out", [rows, OUT], f32, kind="ExternalOutput")

    with tile.TileContext(nc) as tc:
        with (
            tc.tile_pool(name="consts", bufs=1) as cpool,
            tc.tile_pool(name="xin", bufs=4) as xpool,
            tc.tile_pool(name="xbf", bufs=4) as xbpool,
            tc.tile_pool(name="xpad", bufs=4) as xppool,
            tc.tile_pool(name="xt", bufs=4) as xtpool,
            tc.tile_pool(name="h1s", bufs=3) as h1pool,
            tc.tile_pool(name="h2s", bufs=3) as h2pool,
            tc.tile_pool(name="osb", bufs=3) as opool,
            tc.tile_pool(name="ps_h1", bufs=1, space="PSUM") as ps_h1,
            tc.tile_pool(name="ps_h2", bufs=1, space="PSUM") as ps_h2,
            tc.tile_pool(name="ps_o", bufs=1, space="PSUM") as ps_o,
        ):
            a1t = cpool.tile([128, 96], bf16)
            a2t = cpool.tile([96, 128], bf16)
            wpt = cpool.tile([128, 64], bf16)
            b1c = cpool.tile([96, 1], f32)
            b2c = cpool.tile([128, 1], f32)
            bpb = cpool.tile([128, 1024], f32)
            nc.sync.dma_start(a1t[:], a1t_d[:])
            nc.sync.dma_start(a2t[:], a2t_d[:])
            nc.sync.dma_start(wpt[:], wpt_d[:])
            nc.sync.dma_start(b1c[:], b1c_d[:])
            nc.sync.dma_start(b2c[:], b2c_d[:])
            nc.sync.dma_start(bpb[:], bpb_d[:])

            for t in range(ntiles):
                n0 = t * TILE_ROWS
                # ---- load + cast + replicate + transpose ----
                x_sb = xpool.tile([128, 128], f32)
                nc.sync.dma_start(
                    x_sb[:],
                    x_d[n0:n0 + TILE_ROWS, :].rearrange("(p r) f -> p (r f)", p=128),
                )
                x_bf = xbpool.tile([128, 128], bf16)
                nc.vector.tensor_copy(x_bf[:], x_sb[:])
                # x_pad[p, 32a+8g+f] = x_bf[p, 8a+f] = x[n0 + 16p + a, f]
                x_pad = xppool.tile([128, 512], bf16)
                rep_ap = (
                    x_bf[:]
                    .rearrange("p (a f) -> p a f", f=8)
                    .unsqueeze(2)
                    .broadcast_to((128, 16, 4, 8))
                )
                nc.gpsimd.tensor_copy(x_pad[:], rep_ap)
                # xt[32q+8g+f, 32a+v] = x[n0 + 512q + 16v + a, f]
                xt = xtpool.tile([128, 512], bf16)
                nc.vector.transpose(xt[:], x_pad[:])

                # ---- conv1: 4 packed K=8 matmuls, one psum bank each ----
                # rhs streams (v outer, a inner) so bank q's col j = row
                # n0 + 512q + j
                h1ps = ps_h1.tile([96, 2048], f32)
                for q in range(4):
                    rhs = xt[32 * q:32 * q + 8, :].rearrange("k (a v) -> k v a", v=32)
                    nc.tensor.matmul(
                        h1ps[:, 512 * q:512 * q + 512],
                        a1t[32 * q:32 * q + 8, :],
                        rhs,
                        tile_position=(32 * q, 0),
                    )
                # ---- relu1 (+b1): ONE contiguous ACT op ----
                h1s = h1pool.tile([96, 2048], bf16)
                nc.scalar.activation(h1s[:], h1ps[:], Relu, bias=b1c[:])

                # ---- conv2: 4 matmuls; the two 1024-row halves of the tile
                # land on col groups 0/64 and run concurrently ----
                h2ps_a = ps_h2.tile([128, 512], f32)
                h2ps_b = ps_h2.tile([128, 512], f32)
                for ps, lo in ((h2ps_a, 0), (h2ps_b, 512)):
                    for h in (0, 1):
                        nc.tensor.matmul(
                            ps[64 * h:64 * h + 64, :],
                            a2t[:, 64 * h:64 * h + 64],
                            h1s[:, 1024 * h + lo:1024 * h + lo + 512],
                            tile_position=(0, 64 * h),
                        )
                # ---- relu2 (+b2): bank A on ACT, bank B on DVE ----
                h2s_a = h2pool.tile([128, 512], bf16)
                h2s_b = h2pool.tile([128, 512], bf16)
                nc.scalar.activation(h2s_a[:], h2ps_a[:], Relu, bias=b2c[:])
                nc.vector.tensor_scalar(
                    h2s_b[:], h2ps_b[:], b2c[:], 0.0, Alu.add, Alu.max
                )

                # ---- linear: 16 chunks of 128 rows; issue (h=0, h=1) chunk
                # pairs adjacently -> concurrent row groups + distinct banks.
                # chunk c covers rows [n0+128c, +128); h = c//8 selects the
                # h2 partition half, bank = c//8 too (cols 64c).
                outps = ps_o.tile([128, 1024], f32)
                for cc in range(8):
                    for h in (0, 1):
                        c = 8 * h + cc
                        X = (c // 4) % 2
                        h2s = h2s_a if X == 0 else h2s_b
                        col = 128 * (c % 4)
                        nc.tensor.matmul(
                            outps[:, 64 * c:64 * c + 64],
                            h2s[64 * h:64 * h + 64, col:col + 128],
                            wpt[64 * h:64 * h + 64, :],
                            start=(cc == 0),
                            stop=(cc == 7),
                            tile_position=(64 * h, 0),
                        )
                # ---- bias + store ----
                out_sb = opool.tile([128, 1024], f32)
                nc.vector.tensor_tensor(out_sb[:], outps[:], bpb[:], Alu.add)
                nc.sync.dma_start(
                    out_d[n0:n0 + TILE_ROWS, :].rearrange("(c p) j -> p c j", p=128),
                    out_sb[:],
                )

    nc.compile()
    return nc


# ---------------------------------------------------------------------------
# entry point
# ---------------------------------------------------------------------------

_CACHE = {}


def _get_nc(rows=ROWS_CORE):
    if rows not in _CACHE:
        _CACHE[rows] = build_nc(rows)
    return _CACHE[rows]


def make_in_maps(inputs):
    x = np.ascontiguousarray(
        np.asarray(inputs["x"], np.float32)
    ).reshape(ROWS_TOTAL, FEAT)
    consts = pack_weights(
        inputs["W1"], inputs["b1"], inputs["W2"], inputs["b2"],
        inputs["Wp"], inputs["bp"],
    )
    in_maps = []
    for c in range(NCORES):
        m = dict(consts)
        m["x"] = x[c * ROWS_CORE:(c + 1) * ROWS_CORE]
        in_maps.append(m)
    return in_maps


def finish_output(results):
    out = np.concatenate([r["out"] for r in results], axis=0)
    return out.reshape(B, S, OUT).astype(np.float32, copy=False)


def kernel(x, W1, b1, W2, b2, Wp, bp):
    from concourse.bass_utils import run_bass_kernel_spmd

    nc = _get_nc()
    in_maps = make_in_maps(
        {"x": x, "W1": W1, "b1": b1, "W2": W2, "b2": b2, "Wp": Wp, "bp": bp}
    )
    res = run_bass_kernel_spmd(nc, in_maps, core_ids=list(range(NCORES)))
    return finish_output(res.results)



# revision 3
# speedup vs baseline: 1.0332x; 1.0332x over previous
"""Trainium2 Bass kernel for nn_CNNStateEncoder (dense_cnn).

Network per row (B*S rows, 8 features each):
  conv1 2x2 on [1,2,4] -> 32ch x [1,3]   == h1[96]  = A1[96,8]  @ x[8],  relu(+b1)
  conv2 1x2 on [32,1,3] -> 32ch x [1,2]  == h2[64]  = A2[64,96] @ h1,    relu(+b2)
  linear 64->64                          == out[64] = Wp[64,64] @ h2 + bp

v2 mapping (data parallel over 8 cores, 65536 rows/core, 2048-row tiles):
  - input pre-transposed + bf16-cast on HOST -> device loads [8, rows]
    contiguous lines; no on-device cast/replicate/transpose at all
  - feature-major end to end: rows live in the matmul free dim for all
    three layers; the linear keeps Wp stationary so out-features land on
    PSUM partitions and the bias is per-partition
  - PSUM budget/tile: h1 [96,2048] banks0-3, h2 [128,1024] banks4-5,
    out [128,1024] banks6-7 (exactly 8)
  - drains split ACT/DVE at bank boundaries:
      ACT: relu1 cols 0-1024 (banks0-1) + relu2 (banks4-5)
      DVE: relu1 cols 1024-2048 (banks2-3) + out bias-add (banks6-7)
  - output stored bf16 transposed [128, rows/2]; host re-lays + upcasts
  - dma_start issue costs ~600ns on SyncE -> input batched 4 tiles/DMA,
    output batched 2 tiles/DMA
"""

import numpy as np
import ml_dtypes

B, S, FEAT, OUT = 64, 8192, 8, 64
NCORES = 8
ROWS_TOTAL = B * S
ROWS_CORE = ROWS_TOTAL // NCORES  # 65536
TILE_ROWS = 2048
NTILES = ROWS_CORE // TILE_ROWS   # 32
IN_BATCH = 4                      # tiles per input dma
OUT_BATCH = 2                     # tiles per output dma

BF16 = ml_dtypes.bfloat16

# ---------------------------------------------------------------------------
# numpy-side packing
# ---------------------------------------------------------------------------

def pack_weights(W1, b1, W2, b2, Wp, bp):
    W1 = np.asarray(W1, np.float32)
    W2 = np.asarray(W2, np.float32)
    Wp = np.asarray(Wp, np.float32)
    b1 = np.asarray(b1, np.float32)
    b2 = np.asarray(b2, np.float32)
    bp = np.asarray(bp, np.float32)

    # A1 [96, 8]: h1[o*3+j] = sum_{kh,kw} x[kh*4 + j + kw] * W1[o,0,kh,kw]
    A1 = np.zeros((96, 8), np.float32)
    for o in range(32):
        for j in range(3):
            for kh in range(2):
                for kw in range(2):
                    A1[o * 3 + j, kh * 4 + j + kw] += W1[o, 0, kh, kw]
    b1_96 = np.repeat(b1, 3).astype(np.float32)

    # A2 [64, 96]: h2[c*2+w] = sum_{i,kw} h1[i*3 + w + kw] * W2[c,i,0,kw]
    A2 = np.zeros((64, 96), np.float32)
    for c in range(32):
        for w in range(2):
            for i in range(32):
                for kw in range(2):
                    A2[c * 2 + w, i * 3 + w + kw] += W2[c, i, 0, kw]
    b2_64 = np.repeat(b2, 2).astype(np.float32)

    # conv1 stationary at PE row groups 0 and 32 (alternating for LDW overlap)
    a1t = np.zeros((64, 96), np.float32)
    a1t[0:8, :] = A1.T
    a1t[32:40, :] = A1.T
    # conv2 stationary: A2.T replicated on both output col groups
    a2t = np.zeros((96, 128), np.float32)
    a2t[:, 0:64] = A2.T
    a2t[:, 64:128] = A2.T
    # linear stationary: Wp.T on both partition halves (K=64 each)
    wpt = np.zeros((128, 64), np.float32)
    wpt[0:64, :] = Wp.T
    wpt[64:128, :] = Wp.T
    b1c = b1_96.reshape(96, 1)
    b2c = np.concatenate([b2_64, b2_64]).reshape(128, 1)
    bpc = np.concatenate([bp, bp]).reshape(128, 1)

    return {
        "a1t": a1t.astype(BF16),
        "a2t": a2t.astype(BF16),
        "wpt": wpt.astype(BF16),
        "b1c": b1c,
        "b2c": b2c,
        "bpc": bpc,
    }


# ---------------------------------------------------------------------------
# bass module
# ---------------------------------------------------------------------------

def build_nc(rows=ROWS_CORE):
    import concourse.bass as bass
    import concourse.bacc as bacc
    import concourse.mybir as mybir
    import concourse.tile as tile

    f32 = mybir.dt.float32
    bf16 = mybir.dt.bfloat16
    Relu = mybir.ActivationFunctionType.Relu
    Alu = mybir.AluOpType

    ntiles = rows // TILE_ROWS
    chunk = IN_BATCH * TILE_ROWS  # input dma cols

    nc = bacc.Bacc(None, target_bir_lowering=False)

    x_d = nc.dram_tensor("x", [FEAT, rows], bf16, kind="ExternalInput")
    a1t_d = nc.dram_tensor("a1t", [64, 96], bf16, kind="ExternalInput")
    a2t_d = nc.dram_tensor("a2t", [96, 128], bf16, kind="ExternalInput")
    wpt_d = nc.dram_tensor("wpt", [128, 64], bf16, kind="ExternalInput")
    b1c_d = nc.dram_tensor("b1c", [96, 1], f32, kind="ExternalInput")
    b2c_d = nc.dram_tensor("b2c", [128, 1], f32, kind="ExternalInput")
    bpc_d = nc.dram_tensor("bpc", [128, 1], f32, kind="ExternalInput")
    out_d = nc.dram_tensor("out", [128, rows // 2], bf16, kind="ExternalOutput")

    with tile.TileContext(nc) as tc:
        with (
            tc.tile_pool(name="consts", bufs=1) as cpool,
            tc.tile_pool(name="xin", bufs=2) as xpool,
            tc.tile_pool(name="h1s", bufs=2) as h1pool,
            tc.tile_pool(name="h2s", bufs=2) as h2pool,
            tc.tile_pool(name="osb", bufs=2) as opool,
            tc.tile_pool(name="ps_h1", bufs=1, space="PSUM") as ps_h1,
            tc.tile_pool(name="ps_h2", bufs=1, space="PSUM") as ps_h2,
            tc.tile_pool(name="ps_o", bufs=1, space="PSUM") as ps_o,
        ):
            a1t = cpool.tile([64, 96], bf16)
            a2t = cpool.tile([96, 128], bf16)
            wpt = cpool.tile([128, 64], bf16)
            b1c = cpool.tile([96, 1], f32)
            b2c = cpool.tile([128, 1], f32)
            bpc = cpool.tile([128, 1], f32)
            nc.sync.dma_start(a1t[:], a1t_d[:])
            nc.sync.dma_start(a2t[:], a2t_d[:])
            nc.sync.dma_start(wpt[:], wpt_d[:])
            nc.sync.dma_start(b1c[:], b1c_d[:])
            nc.sync.dma_start(b2c[:], b2c_d[:])
            nc.sync.dma_start(bpc[:], bpc_d[:])

            xin = None
            osb = None
            for t in range(ntiles):
                u = t % IN_BATCH
                if u == 0:
                    xin = xpool.tile([64, chunk], bf16)
                    c0 = t * TILE_ROWS
                    # same HBM region into PE row groups 0 and 32
                    nc.sync.dma_start(xin[0:8, :], x_d[:, c0:c0 + chunk])
                    nc.sync.dma_start(xin[32:40, :], x_d[:, c0:c0 + chunk])
                xo = u * TILE_ROWS

                # ---- conv1: 4 MMs N=512, alternating row groups ----
                h1ps = ps_h1.tile([96, 2048], f32)
                for j in range(4):
                    g = 32 * (j % 2)
                    nc.tensor.matmul(
                        h1ps[:, 512 * j:512 * j + 512],
                        a1t[g:g + 8, :],
                        xin[g:g + 8, xo + 512 * j:xo + 512 * j + 512],
                        tile_position=(g, 0),
                    )
                # ---- relu1(+b1): ACT banks0-1, DVE banks2-3 ----
                h1s = h1pool.tile([96, 2048], bf16)
                nc.scalar.activation(
                    h1s[:, 0:1024], h1ps[:, 0:1024], Relu, bias=b1c[:]
                )
                nc.vector.tensor_scalar(
                    h1s[:, 1024:2048], h1ps[:, 1024:2048], b1c[:], 0.0,
                    Alu.add, Alu.max,
                )

                # ---- conv2: 2 col-tiled pairs ----
                h2ps = ps_h2.tile([128, 1024], f32)
                for s in range(2):
                    nc.tensor.matmul(
                        h2ps[0:64, 512 * s:512 * s + 512],
                        a2t[:, 0:64],
                        h1s[:, 1024 * s:1024 * s + 512],
                        tile_position=(0, 0),
                    )
                    nc.tensor.matmul(
                        h2ps[64:128, 512 * s:512 * s + 512],
                        a2t[:, 64:128],
                        h1s[:, 1024 * s + 512:1024 * s + 1024],
                        tile_position=(0, 64),
                    )
                # ---- relu2(+b2): one ACT op ----
                h2s = h2pool.tile([128, 1024], bf16)
                nc.scalar.activation(h2s[:], h2ps[:], Relu, bias=b2c[:])

                # ---- linear: quadrant-packed pairs, Wp stationary ----
                outps = ps_o.tile([128, 1024], f32)
                for s in range(2):
                    nc.tensor.matmul(
                        outps[0:64, 512 * s:512 * s + 512],
                        wpt[0:64, :],
                        h2s[0:64, 512 * s:512 * s + 512],
                        tile_position=(0, 0),
                    )
                    nc.tensor.matmul(
                        outps[64:128, 512 * s:512 * s + 512],
                        wpt[64:128, :],
                        h2s[64:128, 512 * s:512 * s + 512],
                        tile_position=(64, 64),
                    )
                # ---- bias-add + bf16 cast: one DVE op ----
                v = t % OUT_BATCH
                if v == 0:
                    osb = opool.tile([128, OUT_BATCH * 1024], bf16)
                nc.vector.tensor_scalar(
                    osb[:, 1024 * v:1024 * v + 1024], outps[:], bpc[:], None,
                    Alu.add,
                )
                if v == OUT_BATCH - 1:
                    o0 = (t - v) * 1024
                    nc.sync.dma_start(
                        out_d[:, o0:o0 + OUT_BATCH * 1024], osb[:]
                    )

    nc.compile()
    return nc


# ---------------------------------------------------------------------------
# entry point
# ---------------------------------------------------------------------------

_CACHE = {}


def _get_nc(rows=ROWS_CORE):
    if rows not in _CACHE:
        _CACHE[rows] = build_nc(rows)
    return _CACHE[rows]


def make_in_maps(inputs):
    x = np.ascontiguousarray(
        np.asarray(inputs["x"], np.float32)
    ).reshape(ROWS_TOTAL, FEAT)
    xbf = x.astype(BF16)
    consts = pack_weights(
        inputs["W1"], inputs["b1"], inputs["W2"], inputs["b2"],
        inputs["Wp"], inputs["bp"],
    )
    in_maps = []
    for c in range(NCORES):
        m = dict(consts)
        m["x"] = np.ascontiguousarray(
            xbf[c * ROWS_CORE:(c + 1) * ROWS_CORE].T
        )
        in_maps.append(m)
    return in_maps


def finish_output(results):
    # out_d[64u + f, 1024t + 512s + c] = out[2048t + 1024s + 512u + c, f]
    cores = []
    for r in results:
        arr = np.asarray(r["out"]).view(np.uint16)
        arr = arr.reshape(2, 64, NTILES, 2, 512)          # [u, f, t, s, c]
        arr = np.ascontiguousarray(arr.transpose(2, 3, 0, 4, 1))  # [t,s,u,c,f]
        cores.append(arr.reshape(ROWS_CORE, OUT))
    out = np.concatenate(cores, axis=0)
    out = out.view(BF16).astype(np.float32)
    return out.reshape(B, S, OUT)


def kernel(x, W1, b1, W2, b2, Wp, bp):
    from concourse.bass_utils import run_bass_kernel_spmd

    nc = _get_nc()
    in_maps = make_in_maps(
        {"x": x, "W1": W1, "b1": b1, "W2": W2, "b2": b2, "Wp": Wp, "bp": bp}
    )
    res = run_bass_kernel_spmd(nc, in_maps, core_ids=list(range(NCORES)))
    return finish_output(res.results)


# revision 8
# speedup vs baseline: 1.2519x; 1.2116x over previous
"""Trainium2 Bass kernel for nn_CNNStateEncoder (dense_cnn).

Network per row (B*S rows, 8 features each):
  conv1 2x2 on [1,2,4] -> 32ch x [1,3]   == h1[96]  = A1[96,8]  @ x[8],  relu(+b1)
  conv2 1x2 on [32,1,3] -> 32ch x [1,2]  == h2[64]  = A2[64,96] @ h1,    relu(+b2)
  linear 64->64                          == out[64] = Wp[64,64] @ h2 + bp

v2 mapping (data parallel over 8 cores, 65536 rows/core, 2048-row tiles):
  - input pre-transposed + bf16-cast on HOST -> device loads [8, rows]
    contiguous lines; no on-device cast/replicate/transpose at all
  - feature-major end to end: rows live in the matmul free dim for all
    three layers; the linear keeps Wp stationary so out-features land on
    PSUM partitions and the bias is per-partition
  - PSUM budget/tile: h1 [96,2048] banks0-3, h2 [128,1024] banks4-5,
    out [128,1024] banks6-7 (exactly 8)
  - drains split ACT/DVE at bank boundaries:
      ACT: relu1 cols 0-1024 (banks0-1) + relu2 (banks4-5)
      DVE: relu1 cols 1024-2048 (banks2-3) + out bias-add (banks6-7)
  - output stored bf16 transposed [128, rows/2]; host re-lays + upcasts
  - dma_start issue costs ~600ns on SyncE -> input batched 4 tiles/DMA,
    output batched 2 tiles/DMA
"""

import numpy as np
import ml_dtypes

B, S, FEAT, OUT = 64, 8192, 8, 64
NCORES = 8
ROWS_TOTAL = B * S
ROWS_CORE = ROWS_TOTAL // NCORES  # 65536
TILE_ROWS = 2048
NTILES = ROWS_CORE // TILE_ROWS   # 32
IN_BATCH = 4                      # tiles per input dma
OUT_BATCH = 2                     # tiles per output dma

BF16 = ml_dtypes.bfloat16

# ---------------------------------------------------------------------------
# numpy-side packing
# ---------------------------------------------------------------------------

def pack_weights(W1, b1, W2, b2, Wp, bp):
    W1 = np.asarray(W1, np.float32)
    W2 = np.asarray(W2, np.float32)
    Wp = np.asarray(Wp, np.float32)
    b1 = np.asarray(b1, np.float32)
    b2 = np.asarray(b2, np.float32)
    bp = np.asarray(bp, np.float32)

    # A1 [96, 8]: h1[o*3+j] = sum_{kh,kw} x[kh*4 + j + kw] * W1[o,0,kh,kw]
    A1 = np.zeros((96, 8), np.float32)
    for o in range(32):
        for j in range(3):
            for kh in range(2):
                for kw in range(2):
                    A1[o * 3 + j, kh * 4 + j + kw] += W1[o, 0, kh, kw]
    b1_96 = np.repeat(b1, 3).astype(np.float32)

    # A2 [64, 96]: h2[c*2+w] = sum_{i,kw} h1[i*3 + w + kw] * W2[c,i,0,kw]
    A2 = np.zeros((64, 96), np.float32)
    for c in range(32):
        for w in range(2):
            for i in range(32):
                for kw in range(2):
                    A2[c * 2 + w, i * 3 + w + kw] += W2[c, i, 0, kw]
    b2_64 = np.repeat(b2, 2).astype(np.float32)

    # conv1 stationary at PE row groups 0 and 32 (alternating for LDW overlap)
    a1t = np.zeros((64, 96), np.float32)
    a1t[0:8, :] = A1.T
    a1t[32:40, :] = A1.T
    # conv2 stationary: A2.T replicated on both output col groups
    a2t = np.zeros((96, 128), np.float32)
    a2t[:, 0:64] = A2.T
    a2t[:, 64:128] = A2.T
    # linear stationary: Wp.T on both partition halves (K=64 each)
    wpt = np.zeros((128, 64), np.float32)
    wpt[0:64, :] = Wp.T
    wpt[64:128, :] = Wp.T
    b1c = b1_96.reshape(96, 1)
    b2c = np.concatenate([b2_64, b2_64]).reshape(128, 1)
    bpc = np.concatenate([bp, bp]).reshape(128, 1)

    return {
        "a1t": a1t.astype(BF16),
        "a2t": a2t.astype(BF16),
        "wpt": wpt.astype(BF16),
        "b1c": b1c,
        "b2c": b2c,
        "bpc": bpc,
    }


# ---------------------------------------------------------------------------
# bass module
# ---------------------------------------------------------------------------

def build_nc(rows=ROWS_CORE):
    import concourse.bass as bass
    import concourse.bacc as bacc
    import concourse.mybir as mybir
    import concourse.tile as tile

    f32 = mybir.dt.float32
    bf16 = mybir.dt.bfloat16
    Relu = mybir.ActivationFunctionType.Relu
    Alu = mybir.AluOpType

    ntiles = rows // TILE_ROWS
    chunk = IN_BATCH * TILE_ROWS  # input dma cols

    nc = bacc.Bacc(None, target_bir_lowering=False)

    x_d = nc.dram_tensor("x", [FEAT, rows], bf16, kind="ExternalInput")
    a1t_d = nc.dram_tensor("a1t", [64, 96], bf16, kind="ExternalInput")
    a2t_d = nc.dram_tensor("a2t", [96, 128], bf16, kind="ExternalInput")
    wpt_d = nc.dram_tensor("wpt", [128, 64], bf16, kind="ExternalInput")
    b1c_d = nc.dram_tensor("b1c", [96, 1], f32, kind="ExternalInput")
    b2c_d = nc.dram_tensor("b2c", [128, 1], f32, kind="ExternalInput")
    bpc_d = nc.dram_tensor("bpc", [128, 1], f32, kind="ExternalInput")
    out_d = nc.dram_tensor("out", [128, rows // 2], bf16, kind="ExternalOutput")

    with tile.TileContext(nc) as tc:
        with (
            tc.tile_pool(name="consts", bufs=1) as cpool,
            tc.tile_pool(name="xin", bufs=2) as xpool,
            tc.tile_pool(name="h1s", bufs=2) as h1pool,
            tc.tile_pool(name="h2s", bufs=2) as h2pool,
            tc.tile_pool(name="osb", bufs=2) as opool,
            tc.tile_pool(name="ps_h1", bufs=1, space="PSUM") as ps_h1,
            tc.tile_pool(name="ps_h2", bufs=1, space="PSUM") as ps_h2,
            tc.tile_pool(name="ps_o", bufs=1, space="PSUM") as ps_o,
        ):
            a1t = cpool.tile([64, 96], bf16)
            a2t = cpool.tile([96, 128], bf16)
            wpt = cpool.tile([128, 64], bf16)
            b1c = cpool.tile([96, 1], f32)
            b2c = cpool.tile([128, 1], f32)
            bpc = cpool.tile([128, 1], f32)
            nc.sync.dma_start(a1t[:], a1t_d[:])
            nc.sync.dma_start(a2t[:], a2t_d[:])
            nc.sync.dma_start(wpt[:], wpt_d[:])
            nc.sync.dma_start(b1c[:], b1c_d[:])
            nc.sync.dma_start(b2c[:], b2c_d[:])
            nc.sync.dma_start(bpc[:], bpc_d[:])

            # software-pipelined: body t issues conv1/relu1 for tile t+1,
            # out-drain/store for tile t-1, and conv2/relu2/linear for t.
            # Each engine's FIFO then never blocks long:
            #   PE : conv1(t+1), conv2(t), linear(t)
            #   ACT: relu1a(t+1), relu2(t)
            #   DVE: relu1b(t+1), out(t-1)
            state = {}   # tile idx -> dict(h1s=..., outps=...)
            xins = {}    # batch idx -> xin tile (2 in flight)
            osb = [None]

            def load_batch(b):
                if b * IN_BATCH >= ntiles or b in xins:
                    return
                xt = xpool.tile([64, chunk], bf16, name=f"xin{b % 2}")
                c0 = b * IN_BATCH * TILE_ROWS
                # same HBM region into PE row groups 0 and 32
                nc.sync.dma_start(xt[0:8, :], x_d[:, c0:c0 + chunk])
                nc.sync.dma_start(xt[32:40, :], x_d[:, c0:c0 + chunk])
                xins[b] = xt

            def stage_load(i):
                u = i % IN_BATCH
                bi = i // IN_BATCH
                xcur = xins[bi]
                xo = u * TILE_ROWS
                # conv1: 4 MMs N=512, alternating row groups
                h1ps = ps_h1.tile([96, 2048], f32)
                for j in range(4):
                    g = 32 * (j % 2)
                    nc.tensor.matmul(
                        h1ps[:, 512 * j:512 * j + 512],
                        a1t[g:g + 8, :],
                        xcur[g:g + 8, xo + 512 * j:xo + 512 * j + 512],
                        tile_position=(g, 0),
                    )
                if u == IN_BATCH - 1:
                    xins.pop(bi)
                    load_batch(bi + 2)
                # relu1(+b1): ACT banks0-1, DVE banks2-3
                h1s = h1pool.tile([96, 2048], bf16)
                nc.scalar.activation(
                    h1s[:, 0:1024], h1ps[:, 0:1024], Relu, bias=b1c[:]
                )
                nc.vector.tensor_scalar(
                    h1s[:, 1024:2048], h1ps[:, 1024:2048], b1c[:], 0.0,
                    Alu.add, Alu.max,
                )
                state[i] = {"h1s": h1s}

            def stage_mid(i):
                st = state[i]
                h1s = st["h1s"]
                # conv2: 2 col-tiled pairs
                h2ps = ps_h2.tile([128, 1024], f32)
                for s in range(2):
                    nc.tensor.matmul(
                        h2ps[0:64, 512 * s:512 * s + 512],
                        a2t[:, 0:64],
                        h1s[:, 1024 * s:1024 * s + 512],
                        tile_position=(0, 0),
                    )
                    nc.tensor.matmul(
                        h2ps[64:128, 512 * s:512 * s + 512],
                        a2t[:, 64:128],
                        h1s[:, 1024 * s + 512:1024 * s + 1024],
                        tile_position=(0, 64),
                    )
                # relu2(+b2): one ACT op
                h2s = h2pool.tile([128, 1024], bf16)
                nc.scalar.activation(h2s[:], h2ps[:], Relu, bias=b2c[:])
                # linear: quadrant-packed pairs, Wp stationary
                outps = ps_o.tile([128, 1024], f32)
                for s in range(2):
                    nc.tensor.matmul(
                        outps[0:64, 512 * s:512 * s + 512],
                        wpt[0:64, :],
                        h2s[0:64, 512 * s:512 * s + 512],
                        tile_position=(0, 0),
                    )
                    nc.tensor.matmul(
                        outps[64:128, 512 * s:512 * s + 512],
                        wpt[64:128, :],
                        h2s[64:128, 512 * s:512 * s + 512],
                        tile_position=(64, 64),
                    )
                st["outps"] = outps

            def stage_out(i):
                st = state.pop(i)
                v = i % OUT_BATCH
                if v == 0:
                    osb[0] = opool.tile([128, OUT_BATCH * 1024], bf16, name="osb")
                nc.vector.tensor_scalar(
                    osb[0][:, 1024 * v:1024 * v + 1024], st["outps"], bpc[:],
                    None, Alu.add,
                )
                if v == OUT_BATCH - 1:
                    o0 = (i - v) * 1024
                    nc.sync.dma_start(
                        out_d[:, o0:o0 + OUT_BATCH * 1024], osb[0][:]
                    )

            load_batch(0)
            load_batch(1)
            stage_load(0)
            for t in range(ntiles):
                if t + 1 < ntiles:
                    stage_load(t + 1)
                if t >= 1:
                    stage_out(t - 1)
                stage_mid(t)
            stage_out(ntiles - 1)

    nc.compile()
    return nc


# ---------------------------------------------------------------------------
# entry point
# ---------------------------------------------------------------------------

_CACHE = {}


def _get_nc(rows=ROWS_CORE):
    if rows not in _CACHE:
        _CACHE[rows] = build_nc(rows)
    return _CACHE[rows]


def make_in_maps(inputs):
    x = np.ascontiguousarray(
        np.asarray(inputs["x"], np.float32)
    ).reshape(ROWS_TOTAL, FEAT)
    xbf = x.astype(BF16)
    consts = pack_weights(
        inputs["W1"], inputs["b1"], inputs["W2"], inputs["b2"],
        inputs["Wp"], inputs["bp"],
    )
    in_maps = []
    for c in range(NCORES):
        m = dict(consts)
        m["x"] = np.ascontiguousarray(
            xbf[c * ROWS_CORE:(c + 1) * ROWS_CORE].T
        )
        in_maps.append(m)
    return in_maps


def finish_output(results):
    # out_d[64u + f, 1024t + 512s + c] = out[2048t + 1024s + 512u + c, f]
    cores = []
    for r in results:
        arr = np.asarray(r["out"]).view(np.uint16)
        arr = arr.reshape(2, 64, NTILES, 2, 512)          # [u, f, t, s, c]
        arr = np.ascontiguousarray(arr.transpose(2, 3, 0, 4, 1))  # [t,s,u,c,f]
        cores.append(arr.reshape(ROWS_CORE, OUT))
    out = np.concatenate(cores, axis=0)
    out = out.view(BF16).astype(np.float32)
    return out.reshape(B, S, OUT)


def kernel(x, W1, b1, W2, b2, Wp, bp):
    from concourse.bass_utils import run_bass_kernel_spmd

    nc = _get_nc()
    in_maps = make_in_maps(
        {"x": x, "W1": W1, "b1": b1, "W2": W2, "b2": b2, "Wp": Wp, "bp": bp}
    )
    res = run_bass_kernel_spmd(nc, in_maps, core_ids=list(range(NCORES)))
    return finish_output(res.results)


# revision 10
# speedup vs baseline: 1.2962x; 1.0354x over previous
"""Trainium2 Bass kernel for nn_CNNStateEncoder (dense_cnn).

Network per row (B*S rows, 8 features each):
  conv1 2x2 on [1,2,4] -> 32ch x [1,3]   == h1[96]  = A1[96,8]  @ x[8],  relu(+b1)
  conv2 1x2 on [32,1,3] -> 32ch x [1,2]  == h2[64]  = A2[64,96] @ h1,    relu(+b2)
  linear 64->64                          == out[64] = Wp[64,64] @ h2 + bp

v2 mapping (data parallel over 8 cores, 65536 rows/core, 2048-row tiles):
  - input pre-transposed + bf16-cast on HOST -> device loads [8, rows]
    contiguous lines; no on-device cast/replicate/transpose at all
  - feature-major end to end: rows live in the matmul free dim for all
    three layers; the linear keeps Wp stationary so out-features land on
    PSUM partitions and the bias is per-partition
  - PSUM budget/tile: h1 [96,2048] banks0-3, h2 [128,1024] banks4-5,
    out [128,1024] banks6-7 (exactly 8)
  - drains split ACT/DVE at bank boundaries:
      ACT: relu1 cols 0-1024 (banks0-1) + relu2 (banks4-5)
      DVE: relu1 cols 1024-2048 (banks2-3) + out bias-add (banks6-7)
  - output stored bf16 transposed [128, rows/2]; host re-lays + upcasts
  - dma_start issue costs ~600ns on SyncE -> input batched 4 tiles/DMA,
    output batched 2 tiles/DMA
"""

import numpy as np
import ml_dtypes

B, S, FEAT, OUT = 64, 8192, 8, 64
NCORES = 8
ROWS_TOTAL = B * S
ROWS_CORE = ROWS_TOTAL // NCORES  # 65536
TILE_ROWS = 2048
NTILES = ROWS_CORE // TILE_ROWS   # 32
IN_BATCH = 4                      # tiles per input dma
OUT_BATCH = 2                     # tiles per output dma

BF16 = ml_dtypes.bfloat16

# ---------------------------------------------------------------------------
# numpy-side packing
# ---------------------------------------------------------------------------

def pack_weights(W1, b1, W2, b2, Wp, bp):
    W1 = np.asarray(W1, np.float32)
    W2 = np.asarray(W2, np.float32)
    Wp = np.asarray(Wp, np.float32)
    b1 = np.asarray(b1, np.float32)
    b2 = np.asarray(b2, np.float32)
    bp = np.asarray(bp, np.float32)

    # A1 [96, 8]: h1[o*3+j] = sum_{kh,kw} x[kh*4 + j + kw] * W1[o,0,kh,kw]
    A1 = np.zeros((96, 8), np.float32)
    for o in range(32):
        for j in range(3):
            for kh in range(2):
                for kw in range(2):
                    A1[o * 3 + j, kh * 4 + j + kw] += W1[o, 0, kh, kw]
    b1_96 = np.repeat(b1, 3).astype(np.float32)

    # A2 [64, 96]: h2[c*2+w] = sum_{i,kw} h1[i*3 + w + kw] * W2[c,i,0,kw]
    A2 = np.zeros((64, 96), np.float32)
    for c in range(32):
        for w in range(2):
            for i in range(32):
                for kw in range(2):
                    A2[c * 2 + w, i * 3 + w + kw] += W2[c, i, 0, kw]
    b2_64 = np.repeat(b2, 2).astype(np.float32)

    # conv1 stationary at PE row groups 0 and 32 (alternating for LDW overlap)
    a1t = np.zeros((64, 96), np.float32)
    a1t[0:8, :] = A1.T
    a1t[32:40, :] = A1.T
    # conv2 stationary: A2.T replicated on both output col groups
    a2t = np.zeros((96, 128), np.float32)
    a2t[:, 0:64] = A2.T
    a2t[:, 64:128] = A2.T
    # linear stationary: Wp.T on both partition halves (K=64 each)
    wpt = np.zeros((128, 64), np.float32)
    wpt[0:64, :] = Wp.T
    wpt[64:128, :] = Wp.T
    b1c = b1_96.reshape(96, 1)
    b2c = np.concatenate([b2_64, b2_64]).reshape(128, 1)
    bpc = np.concatenate([bp, bp]).reshape(128, 1)

    return {
        "a1t": a1t.astype(BF16),
        "a2t": a2t.astype(BF16),
        "wpt": wpt.astype(BF16),
        "b1c": b1c,
        "b2c": b2c,
        "bpc": bpc,
    }


# ---------------------------------------------------------------------------
# bass module
# ---------------------------------------------------------------------------

def build_nc(rows=ROWS_CORE):
    import concourse.bass as bass
    import concourse.bacc as bacc
    import concourse.mybir as mybir
    import concourse.tile as tile

    f32 = mybir.dt.float32
    bf16 = mybir.dt.bfloat16
    Relu = mybir.ActivationFunctionType.Relu
    Alu = mybir.AluOpType

    ntiles = rows // TILE_ROWS
    chunk = IN_BATCH * TILE_ROWS  # input dma cols

    nc = bacc.Bacc(None, target_bir_lowering=False)

    x_d = nc.dram_tensor("x", [FEAT, rows], bf16, kind="ExternalInput")
    a1t_d = nc.dram_tensor("a1t", [64, 96], bf16, kind="ExternalInput")
    a2t_d = nc.dram_tensor("a2t", [96, 128], bf16, kind="ExternalInput")
    wpt_d = nc.dram_tensor("wpt", [128, 64], bf16, kind="ExternalInput")
    b1c_d = nc.dram_tensor("b1c", [96, 1], f32, kind="ExternalInput")
    b2c_d = nc.dram_tensor("b2c", [128, 1], f32, kind="ExternalInput")
    bpc_d = nc.dram_tensor("bpc", [128, 1], f32, kind="ExternalInput")
    out_d = nc.dram_tensor("out", [128, rows // 2], bf16, kind="ExternalOutput")

    with tile.TileContext(nc) as tc:
        with (
            tc.tile_pool(name="consts", bufs=1) as cpool,
            tc.tile_pool(name="xin", bufs=2) as xpool,
            tc.tile_pool(name="h1s", bufs=2) as h1pool,
            tc.tile_pool(name="h2s", bufs=2) as h2pool,
            tc.tile_pool(name="osb", bufs=2) as opool,
            tc.tile_pool(name="ps_h1", bufs=1, space="PSUM") as ps_h1,
            tc.tile_pool(name="ps_h2", bufs=1, space="PSUM") as ps_h2,
            tc.tile_pool(name="ps_o", bufs=1, space="PSUM") as ps_o,
        ):
            a1t = cpool.tile([64, 96], bf16)
            a2t = cpool.tile([96, 128], bf16)
            wpt = cpool.tile([128, 64], bf16)
            b1c = cpool.tile([96, 1], f32)
            b2c = cpool.tile([128, 1], f32)
            bpc = cpool.tile([128, 1], f32)
            nc.sync.dma_start(a1t[:], a1t_d[:])
            nc.sync.dma_start(a2t[:], a2t_d[:])
            nc.sync.dma_start(wpt[:], wpt_d[:])
            nc.sync.dma_start(b1c[:], b1c_d[:])
            nc.sync.dma_start(b2c[:], b2c_d[:])
            nc.sync.dma_start(bpc[:], bpc_d[:])

            # software-pipelined, stage-skewed 3 deep. Body k issues:
            #   out(k-2) DVE, conv1(k+1) PE, relu1(k+1) ACT+DVE,
            #   conv2(k) PE, relu2(k) ACT, linear(k-1) PE
            # Every same-body RAW flows forward; cross-body deps only
            # reference tiles >=1 body old, so no FIFO-coupled cycle
            # exceeds the per-engine load (DVE 2.46us/tile binds).
            state = {}   # tile idx -> dict(h1s=..., outps=...)
            xins = {}    # batch idx -> xin tile (2 in flight)
            osb = [None]

            def load_batch(b):
                if b * IN_BATCH >= ntiles or b in xins:
                    return
                xt = xpool.tile([64, chunk], bf16, name=f"xin{b % 2}")
                c0 = b * IN_BATCH * TILE_ROWS
                # same HBM region into PE row groups 0 and 32
                nc.sync.dma_start(xt[0:8, :], x_d[:, c0:c0 + chunk])
                nc.sync.dma_start(xt[32:40, :], x_d[:, c0:c0 + chunk])
                xins[b] = xt

            def stage_conv1(i):
                u = i % IN_BATCH
                bi = i // IN_BATCH
                xcur = xins[bi]
                xo = u * TILE_ROWS
                # conv1: 4 MMs N=512, alternating row groups
                h1ps = ps_h1.tile([96, 2048], f32)
                for j in range(4):
                    g = 32 * (j % 2)
                    nc.tensor.matmul(
                        h1ps[:, 512 * j:512 * j + 512],
                        a1t[g:g + 8, :],
                        xcur[g:g + 8, xo + 512 * j:xo + 512 * j + 512],
                        tile_position=(g, 0),
                    )
                if u == IN_BATCH - 1:
                    xins.pop(bi)
                    load_batch(bi + 2)
                state[i] = {"h1ps": h1ps}

            def stage_relu1(i):
                # relu1(+b1): ACT banks0-1, DVE banks2-3
                st = state[i]
                h1ps = st.pop("h1ps")
                h1s = h1pool.tile([96, 2048], bf16)
                nc.scalar.activation(
                    h1s[:, 0:1024], h1ps[:, 0:1024], Relu, bias=b1c[:]
                )
                nc.vector.tensor_scalar(
                    h1s[:, 1024:2048], h1ps[:, 1024:2048], b1c[:], 0.0,
                    Alu.add, Alu.max,
                )
                st["h1s"] = h1s

            def stage_conv2(i):
                st = state[i]
                h1s = st.pop("h1s")
                # conv2: 2 col-tiled pairs
                h2ps = ps_h2.tile([128, 1024], f32)
                for s in range(2):
                    nc.tensor.matmul(
                        h2ps[0:64, 512 * s:512 * s + 512],
                        a2t[:, 0:64],
                        h1s[:, 1024 * s:1024 * s + 512],
                        tile_position=(0, 0),
                    )
                    nc.tensor.matmul(
                        h2ps[64:128, 512 * s:512 * s + 512],
                        a2t[:, 64:128],
                        h1s[:, 1024 * s + 512:1024 * s + 1024],
                        tile_position=(0, 64),
                    )
                # relu2(+b2): one ACT op
                h2s = h2pool.tile([128, 1024], bf16)
                nc.scalar.activation(h2s[:], h2ps[:], Relu, bias=b2c[:])
                st["h2s"] = h2s

            def stage_linear(i):
                st = state[i]
                h2s = st.pop("h2s")
                # linear: quadrant-packed pairs, Wp stationary
                outps = ps_o.tile([128, 1024], f32)
                for s in range(2):
                    nc.tensor.matmul(
                        outps[0:64, 512 * s:512 * s + 512],
                        wpt[0:64, :],
                        h2s[0:64, 512 * s:512 * s + 512],
                        tile_position=(0, 0),
                    )
                    nc.tensor.matmul(
                        outps[64:128, 512 * s:512 * s + 512],
                        wpt[64:128, :],
                        h2s[64:128, 512 * s:512 * s + 512],
                        tile_position=(64, 64),
                    )
                st["outps"] = outps

            def stage_out(i):
                st = state.pop(i)
                v = i % OUT_BATCH
                if v == 0:
                    osb[0] = opool.tile([128, OUT_BATCH * 1024], bf16, name="osb")
                nc.vector.tensor_scalar(
                    osb[0][:, 1024 * v:1024 * v + 1024], st["outps"], bpc[:],
                    None, Alu.add,
                )
                if v == OUT_BATCH - 1:
                    o0 = (i - v) * 1024
                    nc.sync.dma_start(
                        out_d[:, o0:o0 + OUT_BATCH * 1024], osb[0][:]
                    )

            load_batch(0)
            load_batch(1)
            for k in range(-1, ntiles + 2):
                if 2 <= k <= ntiles + 1:
                    stage_out(k - 2)
                if -1 <= k <= ntiles - 2:
                    stage_conv1(k + 1)
                    stage_relu1(k + 1)
                if 0 <= k <= ntiles - 1:
                    stage_conv2(k)
                if 1 <= k <= ntiles:
                    stage_linear(k - 1)

    nc.compile()
    return nc


# ---------------------------------------------------------------------------
# entry point
# ---------------------------------------------------------------------------

_CACHE = {}


def _get_nc(rows=ROWS_CORE):
    if rows not in _CACHE:
        _CACHE[rows] = build_nc(rows)
    return _CACHE[rows]


def make_in_maps(inputs):
    x = np.ascontiguousarray(
        np.asarray(inputs["x"], np.float32)
    ).reshape(ROWS_TOTAL, FEAT)
    xbf = x.astype(BF16)
    consts = pack_weights(
        inputs["W1"], inputs["b1"], inputs["W2"], inputs["b2"],
        inputs["Wp"], inputs["bp"],
    )
    in_maps = []
    for c in range(NCORES):
        m = dict(consts)
        m["x"] = np.ascontiguousarray(
            xbf[c * ROWS_CORE:(c + 1) * ROWS_CORE].T
        )
        in_maps.append(m)
    return in_maps


def finish_output(results):
    # out_d[64u + f, 1024t + 512s + c] = out[2048t + 1024s + 512u + c, f]
    cores = []
    for r in results:
        arr = np.asarray(r["out"]).view(np.uint16)
        arr = arr.reshape(2, 64, NTILES, 2, 512)          # [u, f, t, s, c]
        arr = np.ascontiguousarray(arr.transpose(2, 3, 0, 4, 1))  # [t,s,u,c,f]
        cores.append(arr.reshape(ROWS_CORE, OUT))
    out = np.concatenate(cores, axis=0)
    out = out.view(BF16).astype(np.float32)
    return out.reshape(B, S, OUT)


def kernel(x, W1, b1, W2, b2, Wp, bp):
    from concourse.bass_utils import run_bass_kernel_spmd

    nc = _get_nc()
    in_maps = make_in_maps(
        {"x": x, "W1": W1, "b1": b1, "W2": W2, "b2": b2, "Wp": Wp, "bp": bp}
    )
    res = run_bass_kernel_spmd(nc, in_maps, core_ids=list(range(NCORES)))
    return finish_output(res.results)
